# revision 17
# baseline (speedup 1.0000x reference)
"""Trainium2 Bass kernel for nn_LinearEmbed (GINE message passing + all-pairs edge embed).

Sharding: data-parallel over graphs. 64 graphs -> 8 cores x 8 graphs.
Cross-core coupling: batchnorm statistics (AllReduce of [128,2] per layer).

Layout conventions (per core, G_loc=8 graphs, 512 nodes, 4096 edges):
  feature-major: [H=128 partitions, rows free]  (hT, eT, u1T, A'T, ...)
  edge-major:    [128 edge partitions, H free]  (messages m, ec)
All matmuls in bf16 (f32 PSUM accumulate).

Final stage computes, per graph g and band t (i in [8t,8t+8), 512 pairs):
  out[p] = w2 . relu(A'[i(p)] + B[j(p)] + C[p]) + b2
via PE matmuls: T = B_g @ Rj  +  [ec_band; A'_band] @ Eind  (Eind is a
host-built 0/1 selector, DMA'd per band), then a w2 reduction matmul.
"""

import os
import numpy as np
import ml_dtypes

import concourse.bass as bass
import concourse.mybir as mybir
import concourse.tile as tile
from concourse.vector_clock import ScopedClock
from concourse.bass_utils import run_bass_kernel_spmd

# problem constants
G, NP, EP, H = 64, 64, 512, 128
IN_F, EDGE_F, L = 32, 16, 4
BN_EPS = 1e-5
N_CORES = 8
G_LOC = G // N_CORES          # 8 graphs per core
N_LOC = G_LOC * NP            # 512 nodes
E_LOC = G_LOC * EP            # 4096 edges
NB = G_LOC * 8                # 64 bands per core (8 i-bands per graph)
BS = 96                       # band slot budget (max edges per band)
NTOT = float(G * NP)          # batchnorm population

f32 = mybir.dt.float32
bf16 = mybir.dt.bfloat16
AX = mybir.AxisListType
ALU = mybir.AluOpType
ACTF = mybir.ActivationFunctionType

bf = ml_dtypes.bfloat16


def _to_bf16(a):
    return np.asarray(a, dtype=np.float32).astype(bf)


class _SplitDrainTC(tile.TileContext):
    """Tail drain in this walrus build accepts only one sync wait; split the
    global-clock waits across multiple drain instructions."""

    def _drain_and_barrier(self, tick_clock, wait_clock):
        drain_inst = self.nc.sync.drain()
        wait_clock.add_sem_waits(
            drain_inst.ins, ScopedClock({None: tick_clock.global_clock})
        )
        si = drain_inst.ins.sync_info
        waits = list(si.on_wait or [])
        if len(waits) > 1:
            si.on_wait = [waits[0]]
            for w in waits[1:]:
                extra = self.nc.sync.drain()
                extra.ins.sync_info = mybir.SyncInfo(on_wait=[w], on_update=[])
        self.nc.all_engine_barrier()
        assert self.sems is not None
        popped = self.nc._tile_sem_poison_stack.pop()
        assert popped is self._sem_poison
        self.nc.clear_and_free_semaphores(list(self.sems.allocated().values()))
        self.nc.all_engine_barrier()


# ---------------------------------------------------------------------------
# host-side preprocessing: shard + sort + one ndarray per SBUF constant
# ---------------------------------------------------------------------------

def _prep_core(c, x, edge_attr, src, dst, weights):
    g0 = c * G_LOC
    lo, hi = g0 * NP, (g0 + G_LOC) * NP
    mask = (src >= lo) & (src < hi)
    esel = np.nonzero(mask)[0]
    assert ((dst[esel] >= lo) & (dst[esel] < hi)).all(), "cross-shard edge"

    # stable sort local edges by (graph, band)
    s_loc = src[esel] - lo
    band_key = (s_loc // NP) * 8 + (s_loc % NP) // 8
    order = np.argsort(band_key, kind="stable")
    esel = esel[order]
    s_loc = src[esel] - lo
    d_loc = dst[esel] - lo
    gl = s_loc // NP
    si = s_loc % NP
    di = d_loc % NP
    assert len(esel) == E_LOC, f"core {c}: {len(esel)} edges"
    assert (np.bincount(gl, minlength=G_LOC) == EP).all()

    ea = np.asarray(edge_attr)[esel]          # [E_LOC, 16] sorted order

    # gather matrix (+ ones row for the gbm_b2 bias trick): [65, 8*512]
    gmat = np.zeros((NP + 1, E_LOC), np.float32)
    gmat[si, np.arange(E_LOC)] = 1.0
    gmat[NP, :] = 1.0
    # scatter matrix chunks: [128, 32*64]; chunk (g,ch) -> cols (g*4+ch)*64
    smat = np.zeros((128, E_LOC // 128 * NP), np.float32)
    for g in range(G_LOC):
        for ch in range(EP // 128):
            sel = slice(g * EP + ch * 128, g * EP + (ch + 1) * 128)
            blk = np.zeros((128, NP), np.float32)
            blk[np.arange(128), di[sel]] = 1.0
            smat[:, (g * 4 + ch) * NP:(g * 4 + ch + 1) * NP] = blk

    # banded (padded) edge layout for the final stage; host-built Eind
    eab = np.zeros((NB * BS, EDGE_F), np.float32)
    eib = np.zeros((NB, BS + 9, 512), np.float32)
    eib[:, BS:BS + 8, :] = (np.arange(512) // NP == np.arange(8)[:, None])
    eib[:, BS + 8, :] = 1.0
    bands = gl * 8 + si // 8
    for b in range(NB):
        sel = np.nonzero(bands == b)[0]
        nb = len(sel)
        assert nb <= BS, f"band {b} has {nb} edges > {BS}"
        eab[b * BS:b * BS + nb] = ea[sel]
        eib[b, np.arange(nb), (si[sel] % 8) * NP + di[sel]] = 1.0

    xc = np.asarray(x)[lo:hi]                  # [512, 32]

    w = weights
    out = {
        "xT": _to_bf16(xc.T),                                    # [32, 512]
        "eaT": _to_bf16(ea.T),                                   # [16, 4096]
        "eaTb": _to_bf16(eab.T),                                 # [16, 6144]
        "gmat": _to_bf16(gmat),                                  # [65, 4096]
        "smat": _to_bf16(smat),                                  # [128, 2048]
        "eib": _to_bf16(eib),                                    # [64, 128, 512]
    }
    out.update(w)
    return out


def _prep_shared(atom_W, atom_b, bond_W, bond_b, gbm_W1, gbm_b1, gbm_W2,
                 gbm_b2, gnn_W1, gnn_b1, gnn_W2, gnn_b2, bn_gamma, bn_beta,
                 mlp_W1, mlp_b1, mlp_W2, mlp_b2):
    wsq = np.concatenate([np.asarray(gbm_W1), np.asarray(gbm_W2),
                          np.asarray(gnn_W1), np.asarray(gnn_W2)], 0)  # [16,128,128]
    wsb = np.transpose(wsq, (1, 0, 2)).reshape(H, 16 * H)
    wmlp = np.stack([np.asarray(mlp_W1)[0:128], np.asarray(mlp_W1)[128:256],
                     np.asarray(mlp_W1)[256:384]], 0)                  # [3,128,128]
    wmlp_sb = np.transpose(wmlp, (1, 0, 2)).reshape(H, 3 * H)

    bcols = np.zeros((H, 23), np.float32)
    bcols[:, 0] = np.asarray(atom_b)
    bcols[:, 1] = np.asarray(bond_b)
    bcols[:, 2] = np.asarray(mlp_b1)
    bcols[:, 3:7] = np.asarray(gbm_b1).T
    bcols[:, 7:11] = np.asarray(gnn_b1).T
    bcols[:, 11:15] = np.asarray(gnn_b2).T
    bcols[:, 15:19] = np.asarray(bn_gamma).T
    bcols[:, 19:23] = np.asarray(bn_beta).T

    b2rep = np.tile(np.asarray(gbm_b2), (1, G_LOC))                    # [4, 1024]

    rjx = np.zeros((128, 512), np.float32)
    j_of_p = np.arange(512) % NP
    for n in range(NP):
        rjx[n, j_of_p == n] = 1.0
        rjx[NP + n, j_of_p == n] = 1.0

    b1row = np.tile(np.asarray(mlp_b1)[None, :], (1, G_LOC * 8))       # [1, 8192]

    return {
        "b1row": _to_bf16(b1row),               # [1, 8192]
        "wA": _to_bf16(atom_W),                 # [32, 128]
        "wB": _to_bf16(bond_W),                 # [16, 128]
        "wsb": _to_bf16(wsb),                   # [128, 2048]
        "wmlp": _to_bf16(wmlp_sb),              # [128, 384]
        "w2c": _to_bf16(np.asarray(mlp_W2)),    # [128, 1]
        "bcols": bcols,                         # [128, 23] f32
        "b2rep": _to_bf16(b2rep),               # [4, 1024]
        "rjx": _to_bf16(rjx),                   # [128, 512]
        "ident": _to_bf16(np.eye(128)),         # [128, 128]
    }, float(np.asarray(mlp_b2)[0])


# ---------------------------------------------------------------------------
# device program
# ---------------------------------------------------------------------------

INPUT_SPECS = {
    "xT": ([IN_F, N_LOC], bf16), "eaT": ([EDGE_F, E_LOC], bf16),
    "eaTb": ([EDGE_F, NB * BS], bf16), "gmat": ([NP + 1, E_LOC], bf16),
    "smat": ([128, 32 * NP], bf16), "eib": ([NB, BS + 9, 512], bf16),
    "wA": ([IN_F, H], bf16), "wB": ([EDGE_F, H], bf16),
    "wsb": ([H, 16 * H], bf16), "wmlp": ([H, 3 * H], bf16),
    "w2c": ([H, 1], bf16), "bcols": ([H, 23], f32),
    "b2rep": ([L, G_LOC * H], bf16), "rjx": ([128, 512], bf16),
    "ident": ([128, 128], bf16), "b1row": ([1, NB * H], bf16),
}


def build_program(mlp_b2_val):
    nc = bass.Bass(trn_type="TRN2", num_devices=N_CORES)
    dins = {n: nc.dram_tensor(n, shp, dt, kind="ExternalInput")
            for n, (shp, dt) in INPUT_SPECS.items()}
    y = nc.dram_tensor("y", [NB, 512], f32, kind="ExternalOutput")
    b2dram = dins["b2rep"]

    with _SplitDrainTC(nc) as tc:
        with tc.tile_pool(name="const", bufs=1) as cpool, \
             tc.tile_pool(name="big", bufs=1) as bigp, \
             tc.tile_pool(name="work", bufs=2) as wp, \
             tc.tile_pool(name="fin", bufs=3) as fp, \
             tc.tile_pool(name="dram", bufs=1, space="DRAM") as dram, \
             tc.tile_pool(name="psA", bufs=4, space="PSUM") as psA, \
             tc.tile_pool(name="psS", bufs=2, space="PSUM") as psS, \
             tc.tile_pool(name="psO", bufs=2, space="PSUM") as psO:

            # ---- load constants ----
            sb = {}
            for n, (shp, dt) in INPUT_SPECS.items():
                if n in ("eib", "b1row"):
                    continue
                t = cpool.tile(shp, dt, name=f"c_{n}", tag=f"c_{n}")
                nc.sync.dma_start(t[:], dins[n][:])
                sb[n] = t
            bc = sb["bcols"]

            def bcol(i):
                return bc[:, i:i + 1]

            dmy_in = dram.tile([16, 2], f32, name="dmy_in")
            dmy_out = dram.tile([16, 2], f32, name="dmy_out", addr_space="Shared")
            nc.gpsimd.collective_compute(
                "AllReduce", ALU.add, replica_groups=[list(range(N_CORES))],
                ins=[dmy_in.opt()], outs=[dmy_out.opt()])

            wsb, wmlp = sb["wsb"], sb["wmlp"]

            def wsq(i):
                return wsb[:, i * H:(i + 1) * H]

            # ---- bond encoder (packed + banded) ----
            eT = bigp.tile([H, E_LOC], bf16, name="eT", tag="eT")
            for j in range(E_LOC // 512):
                ps = psA.tile([128, 512], f32, name="ps_e", tag="psA")
                nc.tensor.matmul(ps[:], sb["wB"][:], sb["eaT"][:, j * 512:(j + 1) * 512])
                nc.vector.tensor_scalar_add(eT[:, j * 512:(j + 1) * 512], ps[:], bcol(1))
            eTb = bigp.tile([H, NB * BS], bf16, name="eTb", tag="eTb")
            for j in range(NB * BS // 512):
                ps = psA.tile([128, 512], f32, name="ps_eb", tag="psA")
                nc.tensor.matmul(ps[:], sb["wB"][:], sb["eaTb"][:, j * 512:(j + 1) * 512])
                nc.scalar.activation(eTb[:, j * 512:(j + 1) * 512], ps[:],
                                     ACTF.Identity, bias=bcol(1))

            # ---- atom encoder ----
            hT = [None] * (L + 1)
            hT[0] = wp.tile([H, N_LOC], bf16, name="hT0", tag="hT", bufs=3)
            ps = psA.tile([128, 512], f32, name="ps_h0", tag="psA")
            nc.tensor.matmul(ps[:], sb["wA"][:], sb["xT"][:])
            nc.vector.tensor_scalar_add(hT[0][:], ps[:], bcol(0))

            # ---- u1[l] = relu(e @ gbm_W1[l] + b1[l]) (emitted with 1-layer lookahead) ----
            u1 = [None] * L

            def emit_u1(l):
                u1[l] = wp.tile([H, E_LOC], bf16, name=f"u1_{l}", tag="u1", bufs=4)
                for j in range(E_LOC // 512):
                    ps = psA.tile([128, 512], f32, name="ps_u1", tag="psA")
                    nc.tensor.matmul(ps[:], wsq(l), eT[:, j * 512:(j + 1) * 512])
                    nc.scalar.activation(u1[l][:, j * 512:(j + 1) * 512], ps[:],
                                         ACTF.Relu, bias=bcol(3 + l))

            for _l in range(L):
                emit_u1(_l)

            # ---- ec rows for all bands (final-stage C path, h-independent) ----
            EC_all = bigp.tile([128, NB * H], bf16, name="EC_all", tag="EC_all")
            nc.sync.dma_start(EC_all[BS + 8:BS + 9, :], dins["b1row"][:])
            for b in range(NB):
                pse = psS.tile([BS, H], f32, name="ps_ec", tag="psS")
                nc.tensor.matmul(pse[:], eTb[:, b * BS:(b + 1) * BS],
                                 wmlp[:, 2 * H:3 * H])
                nc.vector.tensor_copy(EC_all[0:BS, b * H:(b + 1) * H], pse[:])

            # ---- node-major h (+bias row) ----
            h_all = bigp.tile([NP + 1, G_LOC * H], bf16, name="h_all", tag="h_all")

            def update_h_all(hts, l):
                nc.sync.dma_start(h_all[NP:NP + 1, :], b2dram[l:l + 1, :])
                for g in range(G_LOC):
                    pst = psS.tile([NP, H], f32, name="ps_ht", tag="psS")
                    nc.tensor.matmul(pst[:], hts[:, g * NP:(g + 1) * NP], sb["ident"][:])
                    nc.scalar.activation(h_all[0:NP, g * H:(g + 1) * H], pst[:], ACTF.Copy)

            update_h_all(hT[0], 0)

            # ---- GINE layers ----
            for l in range(L):
                # messages, edge-major
                m_sb = wp.tile([128, E_LOC], bf16, name=f"m_{l}", tag="m")
                for g in range(G_LOC):
                    psm = psA.tile([128, 512], f32, name="ps_m", tag="psA")
                    for ch in range(4):
                        e0 = g * EP + ch * 128
                        nc.tensor.matmul(psm[:, ch * H:(ch + 1) * H],
                                         u1[l][:, e0:e0 + 128],
                                         wsq(4 + l), start=True, stop=False)
                        nc.tensor.matmul(psm[:, ch * H:(ch + 1) * H],
                                         sb["gmat"][0:NP + 1, e0:e0 + 128],
                                         h_all[0:NP + 1, g * H:(g + 1) * H],
                                         start=False, stop=True)
                    for ch in range(4):
                        dst_ap = m_sb[:, (g * 4 + ch) * H:(g * 4 + ch + 1) * H]
                        src_ap = psm[:, ch * H:(ch + 1) * H]
                        if ch % 2 == 0:
                            nc.scalar.activation(dst_ap, src_ap, ACTF.Relu)
                        else:
                            nc.vector.tensor_scalar_max(dst_ap, src_ap, 0.0)

                # scatter-aggregate (feature-major out)
                psagg = psA.tile([128, 512], f32, name="ps_agg", tag="psA")
                for g in range(G_LOC):
                    for ch in range(4):
                        nc.tensor.matmul(psagg[:, g * NP:(g + 1) * NP],
                                         m_sb[:, (g * 4 + ch) * H:(g * 4 + ch + 1) * H],
                                         sb["smat"][:, (g * 4 + ch) * NP:(g * 4 + ch + 1) * NP],
                                         start=(ch == 0), stop=(ch == 3))
                zT = wp.tile([H, N_LOC], bf16, name=f"zT_{l}", tag="zT")
                nc.vector.tensor_tensor(zT[:], hT[l][:], psagg[:], ALU.add)

                # node MLP
                ps1 = psA.tile([128, 512], f32, name="ps_z1", tag="psA")
                nc.tensor.matmul(ps1[:], wsq(8 + l), zT[:])
                y1T = wp.tile([H, N_LOC], bf16, name=f"y1_{l}", tag="y1")
                nc.scalar.activation(y1T[:], ps1[:], ACTF.Relu, bias=bcol(7 + l))
                ps2 = psA.tile([128, 512], f32, name="ps_z2", tag="psA")
                nc.tensor.matmul(ps2[:], wsq(12 + l), y1T[:])
                z2T = wp.tile([H, N_LOC], f32, name=f"z2_{l}", tag="z2")
                nc.vector.tensor_scalar_add(z2T[:], ps2[:], bcol(11 + l))

                # batchnorm stats + AllReduce
                st = wp.tile([H, 2], f32, name=f"st_{l}", tag="st")
                nc.vector.reduce_sum(st[:, 0:1], z2T[:], axis=AX.X)
                sq = wp.tile([H, N_LOC], f32, name=f"sq_{l}", tag="sq")
                nc.vector.tensor_tensor(sq[:], z2T[:], z2T[:], ALU.mult)
                nc.vector.reduce_sum(st[:, 1:2], sq[:], axis=AX.X)
                cc_in = dram.tile([H, 2], f32, name=f"ccin_{l}")
                cc_out = dram.tile([H, 2], f32, name=f"ccout_{l}", addr_space="Shared")
                nc.sync.dma_start(cc_in[:], st[:])
                nc.gpsimd.collective_compute(
                    "AllReduce", ALU.add,
                    replica_groups=[list(range(N_CORES))],
                    ins=[cc_in.opt()], outs=[cc_out.opt()])
                # keep the PE HAM warm through the AllReduce wait: a chain of
                # throwaway matmuls with no data deps sits at this point of the
                # in-order PE queue and runs exactly during the stall.
                pw = psO.tile([128, 512], f32, name=f"warm_{l}", tag="psO")
                for k in range(24):
                    nc.tensor.matmul(pw[:], wsq(k % 16), eT[:, 0:512])
                st2 = wp.tile([H, 2], f32, name=f"st2_{l}", tag="st2")
                nc.sync.dma_start(st2[:], cc_out[:])

                # alpha = gamma/sqrt(var+eps); beta' = beta - mu*alpha
                s_ = wp.tile([H, 10], f32, name=f"bn_{l}", tag="bn")
                nc.vector.tensor_scalar_mul(s_[:, 0:1], st2[:, 0:1], 1.0 / NTOT)   # mu
                nc.vector.tensor_scalar_mul(s_[:, 1:2], st2[:, 1:2], 1.0 / NTOT)   # E[z^2]
                nc.vector.tensor_tensor(s_[:, 2:3], s_[:, 0:1], s_[:, 0:1], ALU.mult)
                nc.vector.tensor_scalar(s_[:, 3:4], s_[:, 2:3], -1.0, BN_EPS,
                                        ALU.mult, ALU.add)
                nc.vector.tensor_tensor(s_[:, 4:5], s_[:, 3:4], s_[:, 1:2], ALU.add)
                nc.scalar.activation(s_[:, 5:6], s_[:, 4:5], ACTF.Sqrt)
                nc.vector.reciprocal(s_[:, 6:7], s_[:, 5:6])
                nc.vector.tensor_tensor(s_[:, 7:8], s_[:, 6:7], bcol(15 + l), ALU.mult)
                nc.vector.tensor_scalar(s_[:, 8:9], s_[:, 0:1], s_[:, 7:8], -1.0,
                                        ALU.mult, ALU.mult)
                nc.vector.tensor_tensor(s_[:, 9:10], s_[:, 8:9], bcol(19 + l), ALU.add)

                hT[l + 1] = wp.tile([H, N_LOC], bf16, name=f"hT{l + 1}", tag="hT", bufs=3)
                nc.scalar.activation(hT[l + 1][:], z2T[:], ACTF.Relu,
                                     bias=s_[:, 9:10], scale=s_[:, 7:8])
                if l + 1 < L:
                    update_h_all(hT[l + 1], l + 1)

            # ---- final stage ----
            h4 = hT[L]
            # B node-major
            Bn = bigp.tile([128, 512], bf16, name="Bn", tag="Bn")
            for ch in range(4):
                psb = psS.tile([128, H], f32, name="ps_bn", tag="psS")
                nc.tensor.matmul(psb[:], h4[:, ch * H:(ch + 1) * H], wmlp[:, H:2 * H])
                nc.scalar.activation(Bn[:, ch * H:(ch + 1) * H], psb[:], ACTF.Copy)

            for b8 in range(NB // 8):
                g = b8
                # phase A: A' transposes + EC row copies + Eind DMA prefetch
                einds = []
                for i in range(8):
                    b = b8 * 8 + i
                    t = b % 8
                    psa = psS.tile([128, H], f32, name="ps_a8", tag="psS")
                    nc.tensor.matmul(psa[BS:BS + 8, :],
                                     h4[:, g * NP + t * 8:g * NP + t * 8 + 8],
                                     wmlp[:, 0:H], tile_position=(0, 96))
                    nc.vector.tensor_copy(EC_all[BS:BS + 8, b * H:(b + 1) * H],
                                          psa[BS:BS + 8, :])
                    eind = fp.tile([BS + 9, 512], bf16, name="eind", tag="eind",
                                   bufs=9)
                    nc.sync.dma_start(eind[0:BS + 9, :], dins["eib"][b][0:BS + 9, :])
                    einds.append(eind)
                # phase B: dense matmul stream + relus
                rts = []
                for i in range(8):
                    b = b8 * 8 + i
                    pt = psA.tile([128, 512], f32, name="ps_T", tag="psA")
                    rb = (g % 2) * NP
                    nc.tensor.matmul(pt[:], Bn[rb:rb + NP, (g // 2) * H:(g // 2 + 1) * H],
                                     sb["rjx"][rb:rb + NP, :], start=True, stop=False)
                    nc.tensor.matmul(pt[:], EC_all[0:BS + 9, b * H:(b + 1) * H],
                                     einds[i][0:BS + 9, :], start=False, stop=True)
                    relu_t = fp.tile([128, 512], bf16, name="relu_t", tag="relu_t",
                                     bufs=9)
                    if b % 2 == 0:
                        nc.scalar.activation(relu_t[:], pt[:], ACTF.Relu)
                    else:
                        nc.vector.tensor_scalar_max(relu_t[:], pt[:], 0.0)
                    rts.append(relu_t)
                # phase C: w2 reductions (col-tiled groups of 4) + output
                for half in range(2):
                    po = psO.tile([128, 512], f32, name="ps_o", tag="psO")
                    for i in range(4):
                        nc.tensor.matmul(po[32 * i:32 * i + 1, :], sb["w2c"][:],
                                         rts[half * 4 + i][:],
                                         tile_position=(0, 32 * i))
                    stage = fp.tile([128, 512], f32, name="ostage", tag="ostage")
                    nc.scalar.activation(stage[:], po[:], ACTF.Copy, bias=mlp_b2_val)
                    nc.sync.dma_start(y[b8 * 8 + half * 4:b8 * 8 + half * 4 + 4, :],
                                      stage[0:97:32, :])

    _split_multi_waits(nc)
    return nc


def _split_multi_waits(nc, cap=1):
    """This walrus build accepts at most one sync wait per instruction; move
    extra waits onto same-engine NoOps inserted immediately before."""
    for fn in nc.m.functions:
        for bb in fn.blocks:
            out = []
            for inst in bb.instructions:
                si = inst.sync_info
                waits = list(si.on_wait) if si and si.on_wait else []
                if len(waits) > cap:
                    for w in waits[:-cap]:
                        nop = mybir.InstNoOp(
                            name=nc.get_next_instruction_name(),
                            sync_info=mybir.SyncInfo(on_wait=[w], on_update=[]),
                            bass_nofuse=True,
                            engine=inst.engine,
                        )
                        out.append(nop)
                    si.on_wait = waits[-cap:]
                out.append(inst)
            bb.instructions = out


# ---------------------------------------------------------------------------
# entry point
# ---------------------------------------------------------------------------

def kernel(**inputs):
    x = np.asarray(inputs["x"])
    edge_attr = np.asarray(inputs["edge_attr"])
    ei = np.asarray(inputs["edge_index"])
    src, dst = ei[0], ei[1]

    shared, b2val = _prep_shared(
        inputs["atom_W"], inputs["atom_b"], inputs["bond_W"], inputs["bond_b"],
        inputs["gbm_W1"], inputs["gbm_b1"], inputs["gbm_W2"], inputs["gbm_b2"],
        inputs["gnn_W1"], inputs["gnn_b1"], inputs["gnn_W2"], inputs["gnn_b2"],
        inputs["bn_gamma"], inputs["bn_beta"], inputs["mlp_W1"], inputs["mlp_b1"],
        inputs["mlp_W2"], inputs["mlp_b2"])

    in_maps = []
    for c in range(N_CORES):
        m = _prep_core(c, x, edge_attr, src, dst, shared)
        in_maps.append({k: np.ascontiguousarray(v) for k, v in m.items()})

    nc = build_program(b2val)
    trace = bool(int(os.environ.get("KERNEL_TRACE", "0")))
    res = run_bass_kernel_spmd(nc, in_maps, list(range(N_CORES)), trace=trace)
    kernel.last_exec_time_ns = res.exec_time_ns
    kernel.last_trace = res.instructions_and_trace

    out = np.concatenate([res.results[c]["y"].reshape(-1) for c in range(N_CORES)])
    return out.reshape(G * NP * NP, 1).astype(np.float32)


kernel.last_exec_time_ns = None
kernel.last_trace = None


# revision 18
# speedup vs baseline: 1.0650x; 1.0650x over previous
"""Trainium2 Bass kernel for nn_LinearEmbed (GINE message passing + all-pairs edge embed).

Sharding: data-parallel over graphs. 64 graphs -> 8 cores x 8 graphs.
Cross-core coupling: batchnorm statistics (AllReduce of [128,2] per layer).

Layout conventions (per core, G_loc=8 graphs, 512 nodes, 4096 edges):
  feature-major: [H=128 partitions, rows free]  (hT, eT, u1T, A'T, ...)
  edge-major:    [128 edge partitions, H free]  (messages m, ec)
All matmuls in bf16 (f32 PSUM accumulate).

Final stage computes, per graph g and band t (i in [8t,8t+8), 512 pairs):
  out[p] = w2 . relu(A'[i(p)] + B[j(p)] + C[p]) + b2
via PE matmuls: T = B_g @ Rj  +  [ec_band; A'_band] @ Eind  (Eind is a
host-built 0/1 selector, DMA'd per band), then a w2 reduction matmul.
"""

import os
import numpy as np
import ml_dtypes

import concourse.bass as bass
import concourse.mybir as mybir
import concourse.tile as tile
from concourse.vector_clock import ScopedClock
from concourse.bass_utils import run_bass_kernel_spmd

# problem constants
G, NP, EP, H = 64, 64, 512, 128
IN_F, EDGE_F, L = 32, 16, 4
BN_EPS = 1e-5
N_CORES = 8
G_LOC = G // N_CORES          # 8 graphs per core
N_LOC = G_LOC * NP            # 512 nodes
E_LOC = G_LOC * EP            # 4096 edges
NB = G_LOC * 8                # 64 bands per core (8 i-bands per graph)
BS = 96                       # band slot budget (max edges per band)
NTOT = float(G * NP)          # batchnorm population

f32 = mybir.dt.float32
bf16 = mybir.dt.bfloat16
AX = mybir.AxisListType
ALU = mybir.AluOpType
ACTF = mybir.ActivationFunctionType

bf = ml_dtypes.bfloat16


def _to_bf16(a):
    return np.asarray(a, dtype=np.float32).astype(bf)


class _SplitDrainTC(tile.TileContext):
    """Tail drain in this walrus build accepts only one sync wait; split the
    global-clock waits across multiple drain instructions."""

    def _drain_and_barrier(self, tick_clock, wait_clock):
        drain_inst = self.nc.sync.drain()
        wait_clock.add_sem_waits(
            drain_inst.ins, ScopedClock({None: tick_clock.global_clock})
        )
        si = drain_inst.ins.sync_info
        waits = list(si.on_wait or [])
        if len(waits) > 1:
            si.on_wait = [waits[0]]
            for w in waits[1:]:
                extra = self.nc.sync.drain()
                extra.ins.sync_info = mybir.SyncInfo(on_wait=[w], on_update=[])
        self.nc.all_engine_barrier()
        assert self.sems is not None
        popped = self.nc._tile_sem_poison_stack.pop()
        assert popped is self._sem_poison
        self.nc.clear_and_free_semaphores(list(self.sems.allocated().values()))
        self.nc.all_engine_barrier()


# ---------------------------------------------------------------------------
# host-side preprocessing: shard + sort + one ndarray per SBUF constant
# ---------------------------------------------------------------------------

def _prep_core(c, x, edge_attr, src, dst, weights):
    g0 = c * G_LOC
    lo, hi = g0 * NP, (g0 + G_LOC) * NP
    mask = (src >= lo) & (src < hi)
    esel = np.nonzero(mask)[0]
    assert ((dst[esel] >= lo) & (dst[esel] < hi)).all(), "cross-shard edge"

    # stable sort local edges by (graph, band)
    s_loc = src[esel] - lo
    band_key = (s_loc // NP) * 8 + (s_loc % NP) // 8
    order = np.argsort(band_key, kind="stable")
    esel = esel[order]
    s_loc = src[esel] - lo
    d_loc = dst[esel] - lo
    gl = s_loc // NP
    si = s_loc % NP
    di = d_loc % NP
    assert len(esel) == E_LOC, f"core {c}: {len(esel)} edges"
    assert (np.bincount(gl, minlength=G_LOC) == EP).all()

    ea = np.asarray(edge_attr)[esel]          # [E_LOC, 16] sorted order

    # gather matrix (+ ones row for the gbm_b2 bias trick): [65, 8*512]
    gmat = np.zeros((NP + 1, E_LOC), np.float32)
    gmat[si, np.arange(E_LOC)] = 1.0
    gmat[NP, :] = 1.0
    # scatter matrix chunks: [128, 32*64]; chunk (g,ch) -> cols (g*4+ch)*64
    smat = np.zeros((128, E_LOC // 128 * NP), np.float32)
    for g in range(G_LOC):
        for ch in range(EP // 128):
            sel = slice(g * EP + ch * 128, g * EP + (ch + 1) * 128)
            blk = np.zeros((128, NP), np.float32)
            blk[np.arange(128), di[sel]] = 1.0
            smat[:, (g * 4 + ch) * NP:(g * 4 + ch + 1) * NP] = blk

    # banded (padded) edge layout for the final stage; host-built Eind
    eab = np.zeros((NB * BS, EDGE_F), np.float32)
    eib = np.zeros((NB, BS + 9, 512), np.float32)
    eib[:, BS:BS + 8, :] = (np.arange(512) // NP == np.arange(8)[:, None])
    eib[:, BS + 8, :] = 1.0
    bands = gl * 8 + si // 8
    for b in range(NB):
        sel = np.nonzero(bands == b)[0]
        nb = len(sel)
        assert nb <= BS, f"band {b} has {nb} edges > {BS}"
        eab[b * BS:b * BS + nb] = ea[sel]
        eib[b, np.arange(nb), (si[sel] % 8) * NP + di[sel]] = 1.0

    xc = np.asarray(x)[lo:hi]                  # [512, 32]

    w = weights
    out = {
        "xT": _to_bf16(xc.T),                                    # [32, 512]
        "eaT": _to_bf16(ea.T),                                   # [16, 4096]
        "eaTb": _to_bf16(eab.T),                                 # [16, 6144]
        "gmat": _to_bf16(gmat),                                  # [65, 4096]
        "smat": _to_bf16(smat),                                  # [128, 2048]
        "eib": _to_bf16(eib),                                    # [64, 128, 512]
    }
    out.update(w)
    return out


def _prep_shared(atom_W, atom_b, bond_W, bond_b, gbm_W1, gbm_b1, gbm_W2,
                 gbm_b2, gnn_W1, gnn_b1, gnn_W2, gnn_b2, bn_gamma, bn_beta,
                 mlp_W1, mlp_b1, mlp_W2, mlp_b2):
    wsq = np.concatenate([np.asarray(gbm_W1), np.asarray(gbm_W2),
                          np.asarray(gnn_W1), np.asarray(gnn_W2)], 0)  # [16,128,128]
    wsb = np.transpose(wsq, (1, 0, 2)).reshape(H, 16 * H)
    wmlp = np.stack([np.asarray(mlp_W1)[0:128], np.asarray(mlp_W1)[128:256],
                     np.asarray(mlp_W1)[256:384]], 0)                  # [3,128,128]
    wmlp_sb = np.transpose(wmlp, (1, 0, 2)).reshape(H, 3 * H)

    bcols = np.zeros((H, 23), np.float32)
    bcols[:, 0] = np.asarray(atom_b)
    bcols[:, 1] = np.asarray(bond_b)
    bcols[:, 2] = np.asarray(mlp_b1)
    bcols[:, 3:7] = np.asarray(gbm_b1).T
    bcols[:, 7:11] = np.asarray(gnn_b1).T
    bcols[:, 11:15] = np.asarray(gnn_b2).T
    bcols[:, 15:19] = np.asarray(bn_gamma).T
    bcols[:, 19:23] = np.asarray(bn_beta).T

    b2rep = np.tile(np.asarray(gbm_b2), (1, G_LOC))                    # [4, 1024]

    rjx = np.zeros((128, 512), np.float32)
    j_of_p = np.arange(512) % NP
    for n in range(NP):
        rjx[n, j_of_p == n] = 1.0
        rjx[NP + n, j_of_p == n] = 1.0

    b1row = np.tile(np.asarray(mlp_b1)[None, :], (1, G_LOC * 8))       # [1, 8192]

    return {
        "b1row": _to_bf16(b1row),               # [1, 8192]
        "wA": _to_bf16(atom_W),                 # [32, 128]
        "wB": _to_bf16(bond_W),                 # [16, 128]
        "wsb": _to_bf16(wsb),                   # [128, 2048]
        "wmlp": _to_bf16(wmlp_sb),              # [128, 384]
        "w2c": _to_bf16(np.asarray(mlp_W2)),    # [128, 1]
        "bcols": bcols,                         # [128, 23] f32
        "b2rep": _to_bf16(b2rep),               # [4, 1024]
        "rjx": _to_bf16(rjx),                   # [128, 512]
        "ident": _to_bf16(np.eye(128)),         # [128, 128]
    }, float(np.asarray(mlp_b2)[0])


# ---------------------------------------------------------------------------
# device program
# ---------------------------------------------------------------------------

INPUT_SPECS = {
    "xT": ([IN_F, N_LOC], bf16), "eaT": ([EDGE_F, E_LOC], bf16),
    "eaTb": ([EDGE_F, NB * BS], bf16), "gmat": ([NP + 1, E_LOC], bf16),
    "smat": ([128, 32 * NP], bf16), "eib": ([NB, BS + 9, 512], bf16),
    "wA": ([IN_F, H], bf16), "wB": ([EDGE_F, H], bf16),
    "wsb": ([H, 16 * H], bf16), "wmlp": ([H, 3 * H], bf16),
    "w2c": ([H, 1], bf16), "bcols": ([H, 23], f32),
    "b2rep": ([L, G_LOC * H], bf16), "rjx": ([128, 512], bf16),
    "ident": ([128, 128], bf16), "b1row": ([1, NB * H], bf16),
}


def build_program(mlp_b2_val):
    nc = bass.Bass(trn_type="TRN2", num_devices=N_CORES)
    dins = {n: nc.dram_tensor(n, shp, dt, kind="ExternalInput")
            for n, (shp, dt) in INPUT_SPECS.items()}
    y = nc.dram_tensor("y", [NB, 512], f32, kind="ExternalOutput")
    b2dram = dins["b2rep"]

    with _SplitDrainTC(nc) as tc:
        with tc.tile_pool(name="const", bufs=1) as cpool, \
             tc.tile_pool(name="big", bufs=1) as bigp, \
             tc.tile_pool(name="work", bufs=2) as wp, \
             tc.tile_pool(name="fin", bufs=3) as fp, \
             tc.tile_pool(name="dram", bufs=1, space="DRAM") as dram, \
             tc.tile_pool(name="psA", bufs=4, space="PSUM") as psA, \
             tc.tile_pool(name="psS", bufs=2, space="PSUM") as psS, \
             tc.tile_pool(name="psO", bufs=2, space="PSUM") as psO:

            # ---- load constants ----
            sb = {}
            for n, (shp, dt) in INPUT_SPECS.items():
                if n in ("eib", "b1row"):
                    continue
                t = cpool.tile(shp, dt, name=f"c_{n}", tag=f"c_{n}")
                nc.sync.dma_start(t[:], dins[n][:])
                sb[n] = t
            bc = sb["bcols"]

            def bcol(i):
                return bc[:, i:i + 1]

            dmy_in = dram.tile([16, 2], f32, name="dmy_in")
            dmy_out = dram.tile([16, 2], f32, name="dmy_out", addr_space="Shared")
            nc.gpsimd.collective_compute(
                "AllReduce", ALU.add, replica_groups=[list(range(N_CORES))],
                ins=[dmy_in.opt()], outs=[dmy_out.opt()])

            wsb, wmlp = sb["wsb"], sb["wmlp"]

            def wsq(i):
                return wsb[:, i * H:(i + 1) * H]

            # ---- bond encoder (packed + banded) ----
            eT = bigp.tile([H, E_LOC], bf16, name="eT", tag="eT")
            for j in range(E_LOC // 512):
                ps = psA.tile([128, 512], f32, name="ps_e", tag="psA")
                nc.tensor.matmul(ps[:], sb["wB"][:], sb["eaT"][:, j * 512:(j + 1) * 512])
                nc.vector.tensor_scalar_add(eT[:, j * 512:(j + 1) * 512], ps[:], bcol(1))
            eTb = bigp.tile([H, NB * BS], bf16, name="eTb", tag="eTb")
            for j in range(NB * BS // 512):
                ps = psA.tile([128, 512], f32, name="ps_eb", tag="psA")
                nc.tensor.matmul(ps[:], sb["wB"][:], sb["eaTb"][:, j * 512:(j + 1) * 512])
                nc.scalar.activation(eTb[:, j * 512:(j + 1) * 512], ps[:],
                                     ACTF.Identity, bias=bcol(1))

            # ---- atom encoder ----
            hT = [None] * (L + 1)
            hT[0] = wp.tile([H, N_LOC], bf16, name="hT0", tag="hT", bufs=3)
            ps = psA.tile([128, 512], f32, name="ps_h0", tag="psA")
            nc.tensor.matmul(ps[:], sb["wA"][:], sb["xT"][:])
            nc.vector.tensor_scalar_add(hT[0][:], ps[:], bcol(0))

            # ---- u1[l] = relu(e @ gbm_W1[l] + b1[l]) (emitted with 1-layer lookahead) ----
            u1 = [None] * L

            def emit_u1(l):
                u1[l] = wp.tile([H, E_LOC], bf16, name=f"u1_{l}", tag="u1", bufs=4)
                for j in range(E_LOC // 512):
                    ps = psA.tile([128, 512], f32, name="ps_u1", tag="psA")
                    nc.tensor.matmul(ps[:], wsq(l), eT[:, j * 512:(j + 1) * 512])
                    nc.scalar.activation(u1[l][:, j * 512:(j + 1) * 512], ps[:],
                                         ACTF.Relu, bias=bcol(3 + l))

            for _l in range(L):
                emit_u1(_l)

            # ---- ec rows for all bands (final-stage C path, h-independent) ----
            EC_all = bigp.tile([128, NB * H], bf16, name="EC_all", tag="EC_all")
            nc.sync.dma_start(EC_all[BS + 8:BS + 9, :], dins["b1row"][:])
            for b in range(NB):
                pse = psS.tile([BS, H], f32, name="ps_ec", tag="psS")
                nc.tensor.matmul(pse[:], eTb[:, b * BS:(b + 1) * BS],
                                 wmlp[:, 2 * H:3 * H])
                nc.vector.tensor_copy(EC_all[0:BS, b * H:(b + 1) * H], pse[:])

            # ---- node-major h (+bias row) ----
            h_all = bigp.tile([NP + 1, G_LOC * H], bf16, name="h_all", tag="h_all")

            def update_h_all(hts, l):
                nc.sync.dma_start(h_all[NP:NP + 1, :], b2dram[l:l + 1, :])
                for g in range(G_LOC):
                    pst = psS.tile([NP, H], f32, name="ps_ht", tag="psS")
                    nc.tensor.matmul(pst[:], hts[:, g * NP:(g + 1) * NP], sb["ident"][:])
                    nc.scalar.activation(h_all[0:NP, g * H:(g + 1) * H], pst[:], ACTF.Copy)

            update_h_all(hT[0], 0)

            # ---- GINE layers ----
            for l in range(L):
                # messages, edge-major
                m_sb = wp.tile([128, E_LOC], bf16, name=f"m_{l}", tag="m")
                for g in range(G_LOC):
                    psm = psA.tile([128, 512], f32, name="ps_m", tag="psA")
                    for ch in range(4):
                        e0 = g * EP + ch * 128
                        nc.tensor.matmul(psm[:, ch * H:(ch + 1) * H],
                                         u1[l][:, e0:e0 + 128],
                                         wsq(4 + l), start=True, stop=False)
                        nc.tensor.matmul(psm[:, ch * H:(ch + 1) * H],
                                         sb["gmat"][0:NP + 1, e0:e0 + 128],
                                         h_all[0:NP + 1, g * H:(g + 1) * H],
                                         start=False, stop=True)
                    for ch in range(4):
                        dst_ap = m_sb[:, (g * 4 + ch) * H:(g * 4 + ch + 1) * H]
                        src_ap = psm[:, ch * H:(ch + 1) * H]
                        if ch % 2 == 0:
                            nc.scalar.activation(dst_ap, src_ap, ACTF.Relu)
                        else:
                            nc.vector.tensor_scalar_max(dst_ap, src_ap, 0.0)

                # scatter-aggregate (feature-major out)
                psagg = psA.tile([128, 512], f32, name="ps_agg", tag="psA")
                for g in range(G_LOC):
                    for ch in range(4):
                        nc.tensor.matmul(psagg[:, g * NP:(g + 1) * NP],
                                         m_sb[:, (g * 4 + ch) * H:(g * 4 + ch + 1) * H],
                                         sb["smat"][:, (g * 4 + ch) * NP:(g * 4 + ch + 1) * NP],
                                         start=(ch == 0), stop=(ch == 3))
                zT = wp.tile([H, N_LOC], bf16, name=f"zT_{l}", tag="zT")
                nc.vector.tensor_tensor(zT[:], hT[l][:], psagg[:], ALU.add)

                # node MLP
                ps1 = psA.tile([128, 512], f32, name="ps_z1", tag="psA")
                nc.tensor.matmul(ps1[:], wsq(8 + l), zT[:])
                y1T = wp.tile([H, N_LOC], bf16, name=f"y1_{l}", tag="y1")
                nc.scalar.activation(y1T[:], ps1[:], ACTF.Relu, bias=bcol(7 + l))
                ps2 = psA.tile([128, 512], f32, name="ps_z2", tag="psA")
                nc.tensor.matmul(ps2[:], wsq(12 + l), y1T[:])
                z2T = wp.tile([H, N_LOC], f32, name=f"z2_{l}", tag="z2")
                nc.vector.tensor_scalar_add(z2T[:], ps2[:], bcol(11 + l))

                # batchnorm stats + AllReduce
                st = wp.tile([H, 2], f32, name=f"st_{l}", tag="st")
                nc.vector.reduce_sum(st[:, 0:1], z2T[:], axis=AX.X)
                sq = wp.tile([H, N_LOC], f32, name=f"sq_{l}", tag="sq")
                nc.vector.tensor_tensor(sq[:], z2T[:], z2T[:], ALU.mult)
                nc.vector.reduce_sum(st[:, 1:2], sq[:], axis=AX.X)
                cc_in = dram.tile([H, 2], f32, name=f"ccin_{l}")
                cc_out = dram.tile([H, 2], f32, name=f"ccout_{l}", addr_space="Shared")
                nc.sync.dma_start(cc_in[:], st[:])
                nc.gpsimd.collective_compute(
                    "AllReduce", ALU.add,
                    replica_groups=[list(range(N_CORES))],
                    ins=[cc_in.opt()], outs=[cc_out.opt()])
                st2 = wp.tile([H, 2], f32, name=f"st2_{l}", tag="st2")
                nc.sync.dma_start(st2[:], cc_out[:])

                # alpha = gamma/sqrt(var+eps); beta' = beta - mu*alpha
                s_ = wp.tile([H, 10], f32, name=f"bn_{l}", tag="bn")
                nc.vector.tensor_scalar_mul(s_[:, 0:1], st2[:, 0:1], 1.0 / NTOT)   # mu
                nc.vector.tensor_scalar_mul(s_[:, 1:2], st2[:, 1:2], 1.0 / NTOT)   # E[z^2]
                nc.vector.tensor_tensor(s_[:, 2:3], s_[:, 0:1], s_[:, 0:1], ALU.mult)
                nc.vector.tensor_scalar(s_[:, 3:4], s_[:, 2:3], -1.0, BN_EPS,
                                        ALU.mult, ALU.add)
                nc.vector.tensor_tensor(s_[:, 4:5], s_[:, 3:4], s_[:, 1:2], ALU.add)
                nc.scalar.activation(s_[:, 5:6], s_[:, 4:5], ACTF.Sqrt)
                nc.vector.reciprocal(s_[:, 6:7], s_[:, 5:6])
                nc.vector.tensor_tensor(s_[:, 7:8], s_[:, 6:7], bcol(15 + l), ALU.mult)
                nc.vector.tensor_scalar(s_[:, 8:9], s_[:, 0:1], s_[:, 7:8], -1.0,
                                        ALU.mult, ALU.mult)
                nc.vector.tensor_tensor(s_[:, 9:10], s_[:, 8:9], bcol(19 + l), ALU.add)

                hT[l + 1] = wp.tile([H, N_LOC], bf16, name=f"hT{l + 1}", tag="hT", bufs=3)
                nc.scalar.activation(hT[l + 1][:], z2T[:], ACTF.Relu,
                                     bias=s_[:, 9:10], scale=s_[:, 7:8])
                if l + 1 < L:
                    update_h_all(hT[l + 1], l + 1)

            # ---- final stage ----
            h4 = hT[L]
            # B node-major
            Bn = bigp.tile([128, 512], bf16, name="Bn", tag="Bn")
            for ch in range(4):
                psb = psS.tile([128, H], f32, name="ps_bn", tag="psS")
                nc.tensor.matmul(psb[:], h4[:, ch * H:(ch + 1) * H], wmlp[:, H:2 * H])
                nc.scalar.activation(Bn[:, ch * H:(ch + 1) * H], psb[:], ACTF.Copy)

            for b8 in range(NB // 8):
                g = b8
                # phase A: A' transposes + EC row copies + Eind DMA prefetch
                einds = []
                for i in range(8):
                    b = b8 * 8 + i
                    t = b % 8
                    psa = psS.tile([128, H], f32, name="ps_a8", tag="psS")
                    nc.tensor.matmul(psa[BS:BS + 8, :],
                                     h4[:, g * NP + t * 8:g * NP + t * 8 + 8],
                                     wmlp[:, 0:H], tile_position=(0, 96))
                    nc.vector.tensor_copy(EC_all[BS:BS + 8, b * H:(b + 1) * H],
                                          psa[BS:BS + 8, :])
                    eind = fp.tile([BS + 9, 512], bf16, name="eind", tag="eind",
                                   bufs=9)
                    nc.sync.dma_start(eind[0:BS + 9, :], dins["eib"][b][0:BS + 9, :])
                    einds.append(eind)
                # phase B: dense matmul stream + relus
                rts = []
                for i in range(8):
                    b = b8 * 8 + i
                    pt = psA.tile([128, 512], f32, name="ps_T", tag="psA")
                    rb = (g % 2) * NP
                    nc.tensor.matmul(pt[:], Bn[rb:rb + NP, (g // 2) * H:(g // 2 + 1) * H],
                                     sb["rjx"][rb:rb + NP, :], start=True, stop=False)
                    nc.tensor.matmul(pt[:], EC_all[0:BS + 9, b * H:(b + 1) * H],
                                     einds[i][0:BS + 9, :], start=False, stop=True)
                    relu_t = fp.tile([128, 512], bf16, name="relu_t", tag="relu_t",
                                     bufs=9)
                    if b % 2 == 0:
                        nc.scalar.activation(relu_t[:], pt[:], ACTF.Relu)
                    else:
                        nc.vector.tensor_scalar_max(relu_t[:], pt[:], 0.0)
                    rts.append(relu_t)
                # phase C: w2 reductions (col-tiled groups of 4) + output
                for half in range(2):
                    po = psO.tile([128, 512], f32, name="ps_o", tag="psO")
                    for i in range(4):
                        nc.tensor.matmul(po[32 * i:32 * i + 1, :], sb["w2c"][:],
                                         rts[half * 4 + i][:],
                                         tile_position=(0, 32 * i))
                    stage = fp.tile([128, 512], f32, name="ostage", tag="ostage")
                    nc.scalar.activation(stage[:], po[:], ACTF.Copy, bias=mlp_b2_val)
                    nc.sync.dma_start(y[b8 * 8 + half * 4:b8 * 8 + half * 4 + 4, :],
                                      stage[0:97:32, :])

    _split_multi_waits(nc)
    return nc


def _split_multi_waits(nc, cap=1):
    """This walrus build accepts at most one sync wait per instruction; move
    extra waits onto same-engine NoOps inserted immediately before."""
    for fn in nc.m.functions:
        for bb in fn.blocks:
            out = []
            for inst in bb.instructions:
                si = inst.sync_info
                waits = list(si.on_wait) if si and si.on_wait else []
                if len(waits) > cap:
                    for w in waits[:-cap]:
                        nop = mybir.InstNoOp(
                            name=nc.get_next_instruction_name(),
                            sync_info=mybir.SyncInfo(on_wait=[w], on_update=[]),
                            bass_nofuse=True,
                            engine=inst.engine,
                        )
                        out.append(nop)
                    si.on_wait = waits[-cap:]
                out.append(inst)
            bb.instructions = out


# ---------------------------------------------------------------------------
# entry point
# ---------------------------------------------------------------------------

def kernel(**inputs):
    x = np.asarray(inputs["x"])
    edge_attr = np.asarray(inputs["edge_attr"])
    ei = np.asarray(inputs["edge_index"])
    src, dst = ei[0], ei[1]

    shared, b2val = _prep_shared(
        inputs["atom_W"], inputs["atom_b"], inputs["bond_W"], inputs["bond_b"],
        inputs["gbm_W1"], inputs["gbm_b1"], inputs["gbm_W2"], inputs["gbm_b2"],
        inputs["gnn_W1"], inputs["gnn_b1"], inputs["gnn_W2"], inputs["gnn_b2"],
        inputs["bn_gamma"], inputs["bn_beta"], inputs["mlp_W1"], inputs["mlp_b1"],
        inputs["mlp_W2"], inputs["mlp_b2"])

    in_maps = []
    for c in range(N_CORES):
        m = _prep_core(c, x, edge_attr, src, dst, shared)
        in_maps.append({k: np.ascontiguousarray(v) for k, v in m.items()})

    nc = build_program(b2val)
    trace = bool(int(os.environ.get("KERNEL_TRACE", "0")))
    res = run_bass_kernel_spmd(nc, in_maps, list(range(N_CORES)), trace=trace)
    kernel.last_exec_time_ns = res.exec_time_ns
    kernel.last_trace = res.instructions_and_trace

    out = np.concatenate([res.results[c]["y"].reshape(-1) for c in range(N_CORES)])
    return out.reshape(G * NP * NP, 1).astype(np.float32)


kernel.last_exec_time_ns = None
kernel.last_trace = None
